# revision 51
# baseline (speedup 1.0000x reference)
"""CGCoupler Trainium2 Bass kernel.

out[n, ro[k]] += x1[n, r1[k]] * x2[n, r2[k]] * cg[k]  for all k, rows n.

Because the CG index tables address contiguous channel runs, the whole op
decomposes into ~147 contiguous-slice FMAs per row:
    out[:, o:o+d] += c * x1[:, a:a+d] * x2[:, b:b+d]
with d in {32, 64}.  Rows sit on the 128 SBUF partitions, the 640-wide
feature dim on the free axis, and T=8 row-tiles fold into each DVE
instruction via multi-dim access patterns.

All on-chip compute runs in bf16 (tolerance is 2e-2).  The DVE runs
fp32-width ops at 1 elem/cycle but packed bf16 TensorTensor at 2/cycle
(2x_1p perf mode); scalar_tensor_tensor (out += P*c) has no fast mode.  So
contributions sharing an output range and |coefficient| are tree-summed
with TensorTensor adds (2x) into scratch, leaving one 1x STT per group:
    out[o:o+d] (+)= c * (P_i ± P_j ± ...)
First-touch finals are plain scaled copies and run on the otherwise-idle
Activation engine (its own SBUF ports).  DMA runs on the HWDGE rings (the
gpsimd SWDGE path would be descriptor-starved by DVE 2-port ops).

Data-parallel across 8 NeuronCores: each core processes 2048 rows.
"""
import numpy as np

N_CORES = 8
P_DIM = 128
T_FOLD = 8          # row-tiles folded per DVE instruction group

_BUILD_CACHE = {}


def _bf16():
    import ml_dtypes
    return ml_dtypes.bfloat16


# ----------------------------------------------------------------------------
# Planning: decompose index tables into merged slice-op instructions
# ----------------------------------------------------------------------------

def _extract_sliceops(cg, r1, r2, ro):
    M = len(cg)
    ops = []
    k = 0
    while k < M:
        j = k + 1
        while (j < M and r1[j] == r1[j-1] + 1 and r2[j] == r2[j-1] + 1
               and ro[j] == ro[j-1] + 1 and cg[j] == cg[k]):
            j += 1
        ops.append((int(r1[k]), int(r2[k]), int(ro[k]), j - k, float(cg[k])))
        k = j
    return ops


def _build_plan(cg, r1, r2, ro, out_dim):
    """Returns a plan with:
      - prod_instrs: merged 4D TensorTensor products of unique (a,b,d) pairs
        into the P tile.
      - tree_instrs: merged 4D TensorTensor add/sub ops accumulating group
        members into the S (scratch) tile.
      - final_instrs: per (output-range, |c|) group, one TS (first touch,
        scaled copy) or STT (scaled accumulate); 3D APs, merged over
        naturally-contiguous runs.
    """
    ops = _extract_sliceops(cg, r1, r2, ro)

    # unique product pairs -> P-tile slots (first-use order)
    pair_slot, p_size = {}, 0
    for (a, b, o, d, c) in ops:
        key = (a, b, d)
        if key not in pair_slot:
            pair_slot[key] = p_size
            p_size += d

    # products: merge runs with constant (da, db, dslot), equal d (4D TT)
    pair_list = [(a, b, d, s) for (a, b, d), s in pair_slot.items()]
    prod_instrs = []
    i = 0
    while i < len(pair_list):
        a0, b0, d0, s0 = pair_list[i]
        j = i + 1
        da = db = ds = None
        while j < len(pair_list):
            a1, b1, d1, s1 = pair_list[j]
            if d1 != d0:
                break
            pa, pb, _, ps = pair_list[j-1]
            nda, ndb, nds = a1 - pa, b1 - pb, s1 - ps
            if da is None:
                da, db, ds = nda, ndb, nds
            elif (nda, ndb, nds) != (da, db, ds):
                break
            j += 1
        n = j - i
        if n == 1:
            da = db = ds = 0
        ext = max(a0 + max(0, (n - 1) * da) + d0,
                  b0 + max(0, (n - 1) * db) + d0)
        prod_instrs.append(dict(pslot=s0, a=a0, b=b0, d=d0,
                                da=da, db=db, ds=ds, n=n, ext=ext))
        i = j

    # group contributions by (output range, |c|); sign folded into tree ops
    groups = {}
    order = []
    for (a, b, o, d, c) in ops:
        key = (o, d, round(abs(c), 5))
        if key not in groups:
            groups[key] = []
            order.append(key)
        groups[key].append((pair_slot[(a, b, d)], 1.0 if c >= 0 else -1.0))

    # normalized member lists (first member's sign factored out; restored
    # into the final scalar) so shared +-pairs are recognizable across groups
    norm_members = {}
    for key, mem in groups.items():
        sgn0 = mem[0][1]
        norm_members[key] = [(('P', s), sg * sgn0) for s, sg in mem]

    # cross-group CSE: greedily extract item-pair subtrees used by >=2
    # groups into shared nodes N = lo +- hi computed once (items may be
    # P slots or previously extracted nodes)
    node_list = []   # (node_ref, lo_item, hi_item, relsign, d)
    while True:
        cand = {}
        for key, items in norm_members.items():
            d = key[1]
            for i in range(len(items)):
                for j in range(i + 1, len(items)):
                    it0, n0 = items[i]
                    it1, n1 = items[j]
                    k = (min(it0, it1), max(it0, it1), n0 * n1, d)
                    cand[k] = cand.get(k, 0) + 1
        cand = {k: v for k, v in cand.items() if v > 1}
        if not cand:
            break
        lo, hi, rs, d = max(cand, key=lambda k: (cand[k], k))
        node_ref = ('N', len(node_list))
        node_list.append((node_ref, lo, hi, rs, d))
        for key, items in norm_members.items():
            if key[1] != d:
                continue
            ilo = next((i for i, (it, sg) in enumerate(items)
                        if it == lo), None)
            ihi = next((i for i, (it, sg) in enumerate(items)
                        if it == hi), None)
            if ilo is None or ihi is None or ilo == ihi:
                continue
            if items[ilo][1] * items[ihi][1] != rs:
                continue
            sign = items[ilo][1]
            norm_members[key] = ([it for i, it in enumerate(items)
                                  if i not in (ilo, ihi)]
                                 + [(node_ref, sign)])

    # coverage pass decides the single first-touch writer per output range
    # (wide-first order so d=64 writers precede the overlapped d=32 ranges)
    order.sort(key=lambda k: (-k[1], k[2], k[0]))
    covered = np.zeros(out_dim, bool)
    needs_memset = False
    kinds = {}
    for key in order:
        o, d, ac = key
        rng = slice(o, o + d)
        if not covered[rng].any():
            kinds[key] = 'W'            # writer: out = c * S
        else:
            if not covered[rng].all():
                needs_memset = True
            kinds[key] = 'A'            # adder: out += c * S
        covered[rng] = True
    if not covered.all():
        needs_memset = True

    # emission order: all writers first (Activation engine handles them
    # while the DVE keeps streaming), then adders; (c, o) order within each
    # class so scaled-copy/add runs merge.
    order.sort(key=lambda k: (kinds[k] == 'A', k[2], -k[1], k[0]))

    # scratch: S tile holds shared CSE nodes (region at the front) and the
    # per-group tree sums; SS holds Activation-scaled values for adders.
    s_size = 0
    node_off = {}
    for node_ref, lo, hi, rs, d in node_list:
        node_off[node_ref] = s_size
        s_size += d

    def resolve(item):
        t, v = item
        return ('P', v) if t == 'P' else ('S', node_off[item])

    ginfo = []
    for key in order:
        o, d, ac = key
        items = norm_members[key]
        m = len(items)
        sgn0 = groups[key][0][1]
        # re-normalize so the first remaining item enters the sum with +1
        s0 = items[0][1]
        items = [(it, sg * s0) for it, sg in items]
        g = dict(kind=kinds[key], o=o, d=d, c=sgn0 * s0 * ac)
        if m == 1:
            g['src'], g['soff'] = resolve(items[0][0])
        else:
            g['src'] = 'S'
            g['soff'] = s_size
            g['items'] = items
            s_size += d
        ginfo.append(g)

    # tree ops, level-major; level -1 computes the shared CSE nodes, level 0
    # seeds each group sum, level k>=1 accumulates in place.
    raw_trees = []
    for node_ref, lo, hi, rs, d in node_list:
        raw_trees.append(dict(lvl=-1, sslot=node_off[node_ref], d=d,
                              op='add' if rs > 0 else 'subtract',
                              in0=resolve(lo), in1=resolve(hi)))
    max_m = max((len(g.get('items', ())) for g in ginfo), default=0)
    for level in range(max_m - 1):
        lvl_ops = []
        for g in ginfo:
            items = g.get('items')
            if not items or len(items) < level + 2:
                continue
            op = 'add' if items[level + 1][1] > 0 else 'subtract'
            in0 = resolve(items[0][0]) if level == 0 else ('S', g['soff'])
            lvl_ops.append(dict(sslot=g['soff'], d=g['d'], op=op, lvl=level,
                                in0=in0, in1=resolve(items[level + 1][0])))
        lvl_ops.sort(key=lambda t: (t['op'], t['d'], t['in0'], t['in1']))
        raw_trees.extend(lvl_ops)

    # merge tree ops: same level/op/d/src-tiles, constant offset deltas (4D)
    tree_instrs = []
    i = 0
    while i < len(raw_trees):
        t0 = raw_trees[i]
        j = i + 1
        dss = d0 = d1 = None
        while j < len(raw_trees):
            t1, tp = raw_trees[j], raw_trees[j-1]
            if (t1['lvl'] != t0['lvl'] or t1['op'] != t0['op']
                    or t1['d'] != t0['d']
                    or t1['in0'][0] != t0['in0'][0]
                    or t1['in1'][0] != t0['in1'][0]):
                break
            nds = t1['sslot'] - tp['sslot']
            nd0 = t1['in0'][1] - tp['in0'][1]
            nd1 = t1['in1'][1] - tp['in1'][1]
            if dss is None:
                dss, d0, d1 = nds, nd0, nd1
            elif (nds, nd0, nd1) != (dss, d0, d1):
                break
            j += 1
        n = j - i
        if n == 1:
            dss = d0 = d1 = 0
        tree_instrs.append(dict(sslot=t0['sslot'], d=t0['d'], op=t0['op'],
                                in0=t0['in0'], in1=t0['in1'],
                                dss=dss, d0=d0, d1=d1, n=n))
        i = j

    # Activation-engine ops: writers (out = c*src, 3D contiguous runs) and
    # scales (SS = c*src, runs need contiguity in src AND ss)
    writer_instrs = []
    ws = [g for g in ginfo if g['kind'] == 'W']
    i = 0
    while i < len(ws):
        g0 = ws[i]
        j = i + 1
        while j < len(ws):
            g1, gp = ws[j], ws[j-1]
            if (g1['d'] != g0['d'] or g1['c'] != g0['c'] or g1['src'] != g0['src']
                    or g1['o'] - gp['o'] != g0['d']
                    or g1['soff'] - gp['soff'] != g0['d']):
                break
            j += 1
        n = j - i
        writer_instrs.append(dict(src=g0['src'], soff=g0['soff'], o=g0['o'],
                                  c=g0['c'], d=g0['d'], n=n))
        i = j

    # adders in (c, o) order (ginfo order) so same-coefficient scale runs
    # merge; SS slots allocated in the same order so adds merge too.
    ads = [g for g in ginfo if g['kind'] == 'A']
    ss_size = 0
    for g in ads:
        g['ssoff'] = ss_size
        ss_size += g['d']

    scale_instrs = []
    i = 0
    while i < len(ads):
        g0 = ads[i]
        j = i + 1
        while j < len(ads):
            g1, gp = ads[j], ads[j-1]
            if (g1['d'] != g0['d'] or g1['c'] != g0['c'] or g1['src'] != g0['src']
                    or g1['soff'] - gp['soff'] != g0['d']
                    or g1['ssoff'] - gp['ssoff'] != g0['d']):
                break
            j += 1
        n = j - i
        scale_instrs.append(dict(src=g0['src'], soff=g0['soff'], ssoff=g0['ssoff'],
                                 c=g0['c'], d=g0['d'], n=n))
        i = j

    # DVE adds: out += SS, 4D TensorTensor (non-contiguous runs merge when
    # out offsets and SS slots progress at constant strides)
    add_instrs = []
    i = 0
    while i < len(ads):
        g0 = ads[i]
        j = i + 1
        do = dss = None
        while j < len(ads):
            g1, gp = ads[j], ads[j-1]
            if g1['d'] != g0['d']:
                break
            ndo = g1['o'] - gp['o']
            ndss = g1['ssoff'] - gp['ssoff']
            # an instruction must not write the same out range twice (the
            # second write would overwrite, not accumulate) nor overlap
            if abs(ndo) < g0['d'] or ndss == 0:
                break
            if do is None:
                do, dss = ndo, ndss
            elif (ndo, ndss) != (do, dss):
                break
            j += 1
        n = j - i
        if n == 1:
            do = dss = 0
        add_instrs.append(dict(o=g0['o'], ssoff=g0['ssoff'], d=g0['d'],
                               do=do, dss=dss, n=n))
        i = j

    # feature-split point for the group loads (products with operands in
    # [0, H) start after the first half-loads); sim-swept optimum.
    split_h = 256 if any(p['ext'] <= 256 for p in prod_instrs) else 640

    # schedule: interleave Activation writers/scales with the DVE adds that
    # consume them, so neither engine sits on a bulk dependency barrier.
    def spans(base, step, n, d):
        return [(base + k * (step if n > 1 else 0), d) for k in range(n)]

    def overlaps(sp1, sp2):
        return any(a < c + dc and c < a + da for (a, da) in sp1 for (c, dc) in sp2)

    w_spans = [spans(w['o'], w['d'], w['n'], w['d']) for w in writer_instrs]
    s_spans = [spans(s['ssoff'], s['d'], s['n'], s['d']) for s in scale_instrs]
    w_done = [False] * len(writer_instrs)
    s_done = [False] * len(scale_instrs)
    # writers no add ever consumes go FIRST: every later DVE instruction
    # then transitively observes their ticks, so the out-store DMA needs no
    # separate Activation wait (its embedded-wait capacity is 2).
    schedule = []
    any_add_osp = [spans(a['o'], a['do'], a['n'], a['d']) for a in add_instrs]
    for wi in range(len(writer_instrs)):
        if not any(overlaps(w_spans[wi], sp) for sp in any_add_osp):
            schedule.append(('writer', wi)); w_done[wi] = True
    # adds in pairs: both adds' Activation deps go first, so one DVE wait
    # absorber covers the pair (TensorTensor carries no embedded waits)
    for base in range(0, len(add_instrs), 2):
        pair = range(base, min(base + 2, len(add_instrs)))
        for ai in pair:
            a = add_instrs[ai]
            a_osp = spans(a['o'], a['do'], a['n'], a['d'])
            a_ssp = spans(a['ssoff'], a['dss'], a['n'], a['d'])
            for wi in range(len(writer_instrs)):
                if not w_done[wi] and overlaps(w_spans[wi], a_osp):
                    schedule.append(('writer', wi)); w_done[wi] = True
            for si in range(len(scale_instrs)):
                if not s_done[si] and overlaps(s_spans[si], a_ssp):
                    schedule.append(('scale', si)); s_done[si] = True
        for ai in pair:
            schedule.append(('add', ai))
    for wi in range(len(writer_instrs)):
        if not w_done[wi]:
            schedule.append(('writer', wi))
    for si in range(len(scale_instrs)):
        if not s_done[si]:
            schedule.append(('scale', si))

    # output-store split points: emitted back-to-back after the schedule;
    # each chunk's semaphore wait clears as its columns' last touch lands,
    # so earlier chunks' transfers overlap the remaining compute.
    store_cuts = [c for c in (256,) if c < out_dim] + [out_dim]

    return dict(p_size=p_size, s_size=s_size, ss_size=ss_size,
                prod_instrs=prod_instrs, tree_instrs=tree_instrs,
                writer_instrs=writer_instrs, scale_instrs=scale_instrs,
                add_instrs=add_instrs, schedule=schedule, split_h=split_h,
                store_cuts=store_cuts, needs_memset=needs_memset)


# ----------------------------------------------------------------------------
# Bass program
# ----------------------------------------------------------------------------

def _build_bass(plan, rows_per_core, rep_dim, out_dim, repeat=1, compute_repeat=1,
                writers_on_act=False, scales_on_act=True):
    import concourse.bass as bass
    import concourse.mybir as mybir
    from concourse.ap import AP
    from concourse.tile import TileContext
    import concourse.tile as _tile_mod
    from concourse.vector_clock import ScopedClock as _ScopedClock

    # The kernel-tail Drain instruction waits on every proc lane with
    # outstanding ticks, but its CTRL ISA struct only has room for a couple
    # of embedded sync-wait commands ("Too many sync wait commands" in
    # walrus codegen otherwise).  Split the global-clock wait across
    # several Drain instructions, one proc each (waits already observed by
    # the SP engine are elided by add_sem_waits).
    if not getattr(_tile_mod.TileContext, '_cg_drain_patched', False):
        _orig_dab = _tile_mod.TileContext._drain_and_barrier

        def _split_drain_and_barrier(self, tick_clock, wait_clock):
            gc = tick_clock.global_clock
            VC = type(gc)
            procs = []
            for p in range(27):
                t = gc.peek_next(p) - 1
                if t > 0:
                    procs.append((p, t))
            for i in range(0, len(procs), 1):
                pc = VC()
                for p, t in procs[i:i + 1]:
                    for _ in range(t):
                        pc.advance(p)
                d = self.nc.sync.drain()
                wait_clock.add_sem_waits(d.ins, _ScopedClock({None: pc}))
            self.nc.all_engine_barrier()
            popped = self.nc._tile_sem_poison_stack.pop()
            assert popped is self._sem_poison
            self.nc.clear_and_free_semaphores(list(self.sems.allocated().values()))
            self.nc.all_engine_barrier()

        _tile_mod.TileContext._drain_and_barrier = _split_drain_and_barrier
        _tile_mod.TileContext._cg_drain_patched = True

    bf16 = mybir.dt.bfloat16
    T = T_FOLD
    n_groups = rows_per_core // (P_DIM * T)
    assert rows_per_core == n_groups * P_DIM * T
    alu = {'add': mybir.AluOpType.add, 'subtract': mybir.AluOpType.subtract}

    nc = bass.Bass("TRN2")
    x1d = nc.declare_dram_parameter("x1", [rows_per_core, rep_dim], bf16, isOutput=False)
    x2d = nc.declare_dram_parameter("x2", [rows_per_core, rep_dim], bf16, isOutput=False)
    outd = nc.declare_dram_parameter("out", [rows_per_core, out_dim], bf16, isOutput=True)

    def ap_custom(tile, base, dims):
        a = tile[:]
        aplist = [list(a.ap[0])] + [[s, n] for (s, n) in dims]
        return AP(a.tensor, a.offset + base, aplist)

    p_size, s_size = plan['p_size'], plan['s_size']
    ss_size = plan['ss_size']

    with TileContext(nc) as tc:
        with (
            tc.tile_pool(name="io", bufs=2) as iop,
            tc.tile_pool(name="pp", bufs=1) as ppp,
            tc.tile_pool(name="ss", bufs=2) as ssp,
        ):
            def dram_group_ap(dram, g, width):
                # [128p, T, width] view of rows [g*T*128, (g+1)*T*128):
                # row = g*T*128 + t*128 + p, iterated (p, t, f)
                a = dram[:]
                return AP(a.tensor, g * T * P_DIM * width,
                          [[width, P_DIM], [P_DIM * width, T], [1, width]])

            for g in range(n_groups * repeat):
                g = g % n_groups
                X1 = iop.tile([P_DIM, T * rep_dim], bf16, tag="X1")
                X2 = iop.tile([P_DIM, T * rep_dim], bf16, tag="X2")
                O = iop.tile([P_DIM, T * out_dim], bf16, tag="O")
                # feature-split loads on the HWDGE ring (SWDGE descriptor
                # generation runs on the gpsimd Q7s, which the DVE's 2-port
                # ops starve via the shared SBUF port pair): products whose
                # operands lie in the first H features start after the first
                # half-loads, hiding most of the group-0 load latency.
                H = plan['split_h']
                def half_ap(dram, width, lo, hi):
                    a = dram[:]
                    return AP(a.tensor, g * T * P_DIM * width + lo,
                              [[width, P_DIM], [P_DIM * width, T], [1, hi - lo]])
                def x_ap(tile, lo, hi):
                    return ap_custom(tile, lo, [(rep_dim, T), (1, hi - lo)])
                # loads on SWDGE (gpsimd): keeps the HWDGE ring under 8 DMAs
                # per exec so the stores never pick up a sem-lane-reuse wait
                # (the DMA ISA struct holds a single embedded wait).  At
                # load-issue time the DVE is between groups, so SWDGE
                # descriptor generation isn't port-starved.
                nc.gpsimd.dma_start(x_ap(X1, 0, H), half_ap(x1d, rep_dim, 0, H))
                nc.gpsimd.dma_start(x_ap(X2, 0, H), half_ap(x2d, rep_dim, 0, H))
                nc.gpsimd.dma_start(x_ap(X1, H, rep_dim), half_ap(x1d, rep_dim, H, rep_dim))
                nc.gpsimd.dma_start(x_ap(X2, H, rep_dim), half_ap(x2d, rep_dim, H, rep_dim))
                # wait absorbers: 4D-AP TensorTensor instructions cannot
                # carry embedded sync waits (S3S3D3 struct), so soak up the
                # DMA-complete waits with tiny 2D copies first.
                SCR = iop.tile([P_DIM, 8], bf16, tag="SCR")
                SCR2 = iop.tile([P_DIM, 32], bf16, tag="SCR2")
                nc.vector.tensor_copy(SCR[:, 0:2], X1[:, 0:2])
                nc.vector.tensor_copy(SCR[:, 2:4], X2[:, 0:2])
                if plan['needs_memset']:
                    nc.gpsimd.memset(O[:], 0.0)

                for _rep in range(compute_repeat):
                    P = ppp.tile([P_DIM, T * p_size], bf16, tag="P")
                    S = ppp.tile([P_DIM, T * s_size], bf16, tag="S")
                    SS = ssp.tile([P_DIM, T * ss_size], bf16, tag="SS")
                    tiles = {'P': (P, p_size), 'S': (S, s_size)}

                    def emit_product(pi):
                        dims = [(p_size, T), (pi['ds'], pi['n']), (1, pi['d'])]
                        nc.vector.tensor_tensor(
                            ap_custom(P, pi['pslot'], dims),
                            ap_custom(X1, pi['a'],
                                      [(rep_dim, T), (pi['da'], pi['n']), (1, pi['d'])]),
                            ap_custom(X2, pi['b'],
                                      [(rep_dim, T), (pi['db'], pi['n']), (1, pi['d'])]),
                            mybir.AluOpType.mult,
                        )

                    for pi in plan['prod_instrs']:
                        if pi['ext'] <= H:
                            emit_product(pi)
                    # absorb the second half-load completion waits
                    nc.vector.tensor_copy(SCR[:, 4:6], X1[:, H:H+2])
                    nc.vector.tensor_copy(SCR[:, 6:8], X2[:, H:H+2])
                    for pi in plan['prod_instrs']:
                        if pi['ext'] > H:
                            emit_product(pi)
                    for ti in plan['tree_instrs']:
                        t0, off0 = ti['in0']
                        t1, off1 = ti['in1']
                        tile0, w0 = tiles[t0]
                        tile1, w1 = tiles[t1]
                        nc.vector.tensor_tensor(
                            ap_custom(S, ti['sslot'],
                                      [(s_size, T), (ti['dss'], ti['n']), (1, ti['d'])]),
                            ap_custom(tile0, off0,
                                      [(w0, T), (ti['d0'], ti['n']), (1, ti['d'])]),
                            ap_custom(tile1, off1,
                                      [(w1, T), (ti['d1'], ti['n']), (1, ti['d'])]),
                            alu[ti['op']],
                        )
                    # Activation writers/scales interleaved with the DVE
                    # adds that consume them (dependency-ordered schedule).
                    # TensorTensor (S3S3D3) cannot carry embedded sync waits,
                    # so every cross-engine dependency must be observed by a
                    # small TensorCopy first; engines run in order, so one
                    # copy waiting on the LATEST Activation tick covers all
                    # earlier Activation work.
                    last_act = None
                    scr2_off = 0
                    def act_absorber():
                        nonlocal last_act, scr2_off
                        if last_act is None:
                            return
                        what2, k = last_act
                        if what2 == 'writer':
                            wi2 = plan['writer_instrs'][k]
                            src = ap_custom(O, wi2['o'], [(1, 2)])
                        else:
                            si2 = plan['scale_instrs'][k]
                            src = ap_custom(SS, si2['ssoff'], [(1, 2)])
                        nc.vector.tensor_copy(SCR2[:, scr2_off:scr2_off + 2], src)
                        scr2_off = (scr2_off + 2) % 32
                        last_act = None

                    for sched_idx, (what, idx) in enumerate(plan['schedule']):
                        if what == 'writer':
                            wi = plan['writer_instrs'][idx]
                            w = wi['n'] * wi['d']
                            stile, sw = tiles[wi['src']]
                            o_ap = ap_custom(O, wi['o'], [(out_dim, T), (1, w)])
                            s_ap = ap_custom(stile, wi['soff'], [(sw, T), (1, w)])
                            if writers_on_act:
                                nc.scalar.activation(
                                    o_ap, s_ap,
                                    mybir.ActivationFunctionType.Copy,
                                    bias=0.0, scale=float(wi['c']))
                                last_act = (what, idx)
                            else:
                                nc.vector.tensor_scalar_mul(o_ap, s_ap, float(wi['c']))
                        elif what == 'scale':
                            si = plan['scale_instrs'][idx]
                            w = si['n'] * si['d']
                            stile, sw = tiles[si['src']]
                            ss_ap = ap_custom(SS, si['ssoff'], [(ss_size, T), (1, w)])
                            s_ap = ap_custom(stile, si['soff'], [(sw, T), (1, w)])
                            if scales_on_act:
                                nc.scalar.activation(
                                    ss_ap, s_ap,
                                    mybir.ActivationFunctionType.Copy,
                                    bias=0.0, scale=float(si['c']))
                                last_act = (what, idx)
                            else:
                                nc.vector.tensor_scalar_mul(ss_ap, s_ap, float(si['c']))
                        else:
                            act_absorber()
                            ai = plan['add_instrs'][idx]
                            o_ap = ap_custom(O, ai['o'],
                                             [(out_dim, T), (ai['do'], ai['n']), (1, ai['d'])])
                            ss_ap = ap_custom(SS, ai['ssoff'],
                                              [(ss_size, T), (ai['dss'], ai['n']), (1, ai['d'])])
                            nc.vector.tensor_tensor(o_ap, o_ap, ss_ap,
                                                    mybir.AluOpType.add)

                    # observe the group's final Activation tick on the DVE so
                    # the next group's TensorTensor ops (which cannot carry
                    # waits) need no cross-engine waits for tile reuse.
                    act_absorber()

                lo = 0
                for ci, hi in enumerate(plan['store_cuts']):
                    nc.scalar.dma_start(
                        half_ap(outd, out_dim, lo, hi),
                        ap_custom(O, lo, [(out_dim, T), (1, hi - lo)]))
                    lo = hi
    return nc


# ----------------------------------------------------------------------------
# Entry point
# ----------------------------------------------------------------------------

def kernel(x1, x2, cg_tilde, repids_in1, repids_in2, repids_out, out_dim):
    from concourse.bass_utils import run_bass_kernel_spmd

    bf16 = _bf16()
    x1 = np.asarray(x1, dtype=np.float32).astype(bf16)
    x2 = np.asarray(x2, dtype=np.float32).astype(bf16)
    cg = np.asarray(cg_tilde, dtype=np.float32)
    r1 = np.asarray(repids_in1).astype(np.int64)
    r2 = np.asarray(repids_in2).astype(np.int64)
    ro = np.asarray(repids_out).astype(np.int64)
    out_dim = int(out_dim)

    n, rep_dim = x1.shape
    rows_per_core = n // N_CORES

    key = (rows_per_core, rep_dim, out_dim, cg.tobytes(), r1.tobytes(),
           r2.tobytes(), ro.tobytes())
    cache_key = hash(key)
    if cache_key not in _BUILD_CACHE:
        plan = _build_plan(cg, r1, r2, ro, out_dim)
        nc = _build_bass(plan, rows_per_core, rep_dim, out_dim)
        _BUILD_CACHE[cache_key] = nc
    nc = _BUILD_CACHE[cache_key]

    in_maps = [
        {"x1": x1[i*rows_per_core:(i+1)*rows_per_core],
         "x2": x2[i*rows_per_core:(i+1)*rows_per_core]}
        for i in range(N_CORES)
    ]
    res = run_bass_kernel_spmd(nc, in_maps, list(range(N_CORES)))
    out = np.concatenate([res.results[i]["out"] for i in range(N_CORES)], axis=0)
    return out.astype(np.float32)
